# revision 1
# baseline (speedup 1.0000x reference)
"""Self-contained Trainium2 Bass kernel for a 2-layer GAT + BatchNorm + graph pooling.

v3 design (single big collective, replicated dense layers, bf16 gather tables,
batched DMA, collective/compute overlap):
  - Dense layers are computed REPLICATED on every core (full-node matmuls are
    cheap on PE) so the baseline's two 8MB AllGathers disappear.
  - Collectives: one tiny AllReduce (BN stats, 2KB) issued first, then one
    AllGather of the layer-1 edge output g1 ([256, 6250] bf16 per core,
    3.2MB). Own-node work (a_dst2 table, x1 pooling scans) overlaps the
    AllGather; only dense L2 waits for it.
  - Gather-table rows are 768 B: 256 bf16 features + 4 f32 att-src logits.
    Edge-phase selector matmuls and DVE ops run in bf16.
  - Edge phase per 128-dst-node block: dma_gather feature rows by src (lo/hi
    split for int16 indices), per-edge logits z = lrelu(a_src+a_dst) via a
    256B-row gather of a_dst by local dst, softmax-weighted segment sum via
    0/1 selector matmuls accumulating [out | denom] in PSUM.
  - Dense phases batch 8 node-blocks per DMA; PSUM->SBUF copies ride the
    (otherwise idle) Activation engine.
  - Pooling: segmented running sum & max along the node axis per channel tile
    (tensor_tensor_scan); host reads each graph's last column and combines
    the <=2 per-graph partials from adjacent cores.
"""

import numpy as np

import concourse.bass as bass
import concourse.bacc as bacc
import concourse.tile as tile
from concourse import mybir
from concourse import bass_utils
from concourse.masks import make_identity

F32 = mybir.dt.float32
BF16 = mybir.dt.bfloat16
I16 = mybir.dt.int16
ALU = mybir.AluOpType
ACTF = mybir.ActivationFunctionType
NPF32 = mybir.dt.np(F32)
NPBF16 = mybir.dt.np(BF16)

# problem constants (hardcoded per the harness contract)
N, F_IN, C0, C1, H, E, G = 50000, 128, 64, 64, 4, 800000, 256
HC = H * C0            # 256
NEG_SLOPE = 0.2
BN_EPS = 1e-5
NCORES = 8
NPC = N // NCORES      # nodes per core (6250)
SPLIT = 32768          # dma_gather int16 index limit -> split gather table
TW = 384               # gather-table row width in bf16 elems (768 B)
ALW = 64               # a_dst table row width in f32 (256 B)
PART = 128
NB = (NPC + PART - 1) // PART       # local dst blocks per core (49)
NGB = (N + PART - 1) // PART        # global node blocks (391)
DCH = 16                            # node blocks per dense-phase DMA chunk

PHASES = 6   # bisection aid: 1=dense1, 2=+edge1, 3=+coll+BN, 4=+dense2,
             # 5=+edge2, 6=+pooling
SKIP_COLL = False
EDGE_OPS = 3   # 1=gathers only, 3=full


# --------------------------------------------------------------------------
# host-side preprocessing
# --------------------------------------------------------------------------

def _pack16(stream_i16, ncols):
    """dma_gather index layout: position i -> [i%16, i//16], replicated to
    partition groups 16k+p for the 8 Q7 cores."""
    base = stream_i16.reshape(ncols, 16).T          # [16, ncols]
    return np.tile(base, (8, 1)).astype(np.int16)   # [128, ncols]


def preprocess(x, edge_index, batch,
               W1, att_src1, att_dst1, b1, gamma, beta,
               W2, att_src2, att_dst2, b2):
    x = np.asarray(x, np.float32)
    edge_index = np.asarray(edge_index)
    batch = np.asarray(batch).astype(np.int64)
    W1 = np.asarray(W1, np.float32); W2 = np.asarray(W2, np.float32)

    src = np.concatenate([edge_index[0], np.arange(N, dtype=np.int64)])
    dst = np.concatenate([edge_index[1], np.arange(N, dtype=np.int64)])

    blocks = []
    nlo_max = nhi_max = 0
    for r in range(NCORES):
        m = (dst >= r * NPC) & (dst < (r + 1) * NPC)
        s_r = src[m]
        dloc = (dst[m] - r * NPC)
        order = np.argsort(dloc, kind="stable")
        s_r = s_r[order]; dloc = dloc[order]
        blk = dloc // PART
        core_blocks = []
        for b in range(NB):
            bm = blk == b
            sb_ = s_r[bm]; db_ = dloc[bm]
            lo_m = sb_ < SPLIT
            core_blocks.append((sb_[lo_m], sb_[~lo_m] - SPLIT,
                                db_[lo_m], db_[~lo_m]))
            nlo_max = max(nlo_max, int(lo_m.sum()))
            nhi_max = max(nhi_max, int((~lo_m).sum()))
        blocks.append(core_blocks)

    KLO = max(1, (nlo_max + PART - 1) // PART)
    KHI = max(1, (nhi_max + PART - 1) // PART)
    KT = KLO + KHI
    IW = 2 * KT * 8 + KT   # idx_all row width (int16 cols per partition)

    idx_all = np.zeros((NCORES, NB, PART, IW), np.int16)
    for r in range(NCORES):
        for b in range(NB):
            lo_src, hi_src, al_lo, al_hi = blocks[r][b]
            ls = np.zeros(KLO * PART, np.int16); ls[:len(lo_src)] = lo_src
            hs = np.zeros(KHI * PART, np.int16); hs[:len(hi_src)] = hi_src
            als = np.zeros(KT * PART, np.int16)
            als[:len(al_lo)] = al_lo
            als[KLO * PART:KLO * PART + len(al_hi)] = al_hi
            ds = np.full(KT * PART, 999, np.int16)      # within-block dst + pad
            ds[:len(al_lo)] = al_lo % PART
            ds[KLO * PART:KLO * PART + len(al_hi)] = al_hi % PART
            idx_all[r, b, :, 0:KLO * 8] = _pack16(ls, KLO * 8)
            idx_all[r, b, :, KLO * 8:KT * 8] = _pack16(hs, KHI * 8)
            idx_all[r, b, :, KT * 8:2 * KT * 8] = _pack16(als, KT * 8)
            idx_all[r, b, :, 2 * KT * 8:] = ds.reshape(KT, PART).T

    # batch-derived pooling metadata
    counts = np.bincount(batch, minlength=G).astype(np.float64)
    maskrow = np.zeros((NCORES, 1, NPC), np.float32)
    cinvrow = np.zeros((NCORES, 1, NPC), np.float32)
    lastcol = [dict() for _ in range(NCORES)]
    for r in range(NCORES):
        bseg = batch[r * NPC:(r + 1) * NPC]
        same = np.ones(NPC, np.float32)
        same[0] = 0.0
        same[1:] = (bseg[1:] == bseg[:-1]).astype(np.float32)
        maskrow[r, 0] = same
        cinvrow[r, 0] = (1.0 / np.maximum(counts[bseg], 1.0)).astype(np.float32)
        gids, last_idx = np.unique(bseg[::-1], return_index=True)
        for g_, li in zip(gids, last_idx):
            lastcol[r][int(g_)] = NPC - 1 - int(li)

    # weight preprocessing: fold attention vectors into extra matmul columns
    # B = [a_src-proj | a_dst-proj] so [h | a_s | a_d] = x @ [W | B]
    def bmat(W, a_s, a_d, fin):
        Wr = W.reshape(fin, H, C0)
        bs = np.einsum("khc,hc->kh", Wr, np.asarray(a_s, np.float32))
        bd = np.einsum("khc,hc->kh", Wr, np.asarray(a_d, np.float32))
        return np.concatenate([bs, bd], axis=1).astype(np.float32)  # [fin, 8]

    B1 = bmat(W1, att_src1, att_dst1, F_IN)
    B2 = bmat(W2, att_src2, att_dst2, HC)
    Wcat1 = np.concatenate([W1, B1], axis=1).astype(np.float32)     # [128, 264]
    W2cat = np.concatenate([W2, B2], axis=1).astype(NPF32)          # [256, 264]

    shared = dict(
        xTf=np.ascontiguousarray(x.T),
        Wcat1=Wcat1,
        W2cat=W2cat.astype(NPBF16),
        B2f=np.ascontiguousarray(B2).astype(np.float32),            # [256, 8]
        b1row=np.asarray(b1, np.float32).reshape(2, PART),
        b2row=np.asarray(b2, np.float32).reshape(2, PART),
        gcol=np.asarray(gamma, np.float32).reshape(2, PART).T.copy(),
        bcol=np.asarray(beta, np.float32).reshape(2, PART).T.copy(),
    )
    in_maps = []
    for r in range(NCORES):
        in_maps.append(dict(
            shared,
            xT=np.ascontiguousarray(x[r * NPC:(r + 1) * NPC].T),
            idx_all=idx_all[r],
            maskrow=maskrow[r],
            cinvrow=cinvrow[r],
        ))
    meta = dict(NB=NB, KLO=KLO, KHI=KHI, KT=KT, lastcol=lastcol, counts=counts)
    return in_maps, meta


# --------------------------------------------------------------------------
# device program
# --------------------------------------------------------------------------

def build_program(meta):
    KLO, KHI, KT = meta["KLO"], meta["KHI"], meta["KT"]
    IW = 2 * KT * 8 + KT
    nc = bacc.Bacc("TRN2", target_bir_lowering=False, debug=False,
                   num_devices=NCORES)

    def ein(name, shape, dt=F32):
        return nc.dram_tensor(name, list(shape), dt, kind="ExternalInput").ap()

    xTf_d = ein("xTf", [F_IN, N])
    xT_d = ein("xT", [F_IN, NPC])
    Wcat1_d = ein("Wcat1", [F_IN, 264])
    W2cat_d = ein("W2cat", [HC, 264], BF16)
    B2f_d = ein("B2f", [HC, 8])
    b1r_d = ein("b1row", [2, PART]); b2r_d = ein("b2row", [2, PART])
    gcol_d = ein("gcol", [PART, 2]); bcol_d = ein("bcol", [PART, 2])
    iall_d = ein("idx_all", [NB, PART, IW], I16)
    mask_d = ein("maskrow", [1, NPC])
    cinv_d = ein("cinvrow", [1, NPC])

    omax_d = nc.dram_tensor("out_max", [4 * PART, NPC], F32, kind="ExternalOutput").ap()
    omean_d = nc.dram_tensor("out_mean", [4 * PART, NPC], F32, kind="ExternalOutput").ap()

    # internal DRAM
    T1 = nc.dram_tensor("T1", [N, TW], BF16).ap()
    T2 = nc.dram_tensor("T2", [N, TW], BF16).ap()
    al1 = nc.dram_tensor("al1", [NB * PART, ALW], F32).ap()
    al2 = nc.dram_tensor("al2", [NB * PART, ALW], F32).ap()
    g1T = nc.dram_tensor("g1T", [HC, NPC], F32).ap()
    x2T = nc.dram_tensor("x2T", [HC, NPC], F32).ap()
    agin = nc.dram_tensor("agin", [HC, NPC], BF16).ap()
    Gt_d = nc.dram_tensor("Gt", [NCORES * HC, NPC], BF16, addr_space="Shared").ap()
    ar_in = nc.dram_tensor("ar_in", [PART, 4], F32).ap()
    ar_out = nc.dram_tensor("ar_out", [PART, 4], F32, addr_space="Shared").ap()

    rgroups = [list(range(NCORES))]

    class _PhaseStopE(Exception):
        pass

    with tile.TileContext(nc) as tc:
      try:
        with tc.tile_pool(name="const", bufs=1) as cpool:
            ident = cpool.tile([PART, PART], F32)
            make_identity(nc, ident[:])
            iota_i = cpool.tile([PART, PART], mybir.dt.int32)
            nc.gpsimd.iota(iota_i[:], pattern=[[1, PART]], base=0,
                           channel_multiplier=0)
            iota_f = cpool.tile([PART, PART], F32)
            nc.vector.tensor_copy(out=iota_f[:], in_=iota_i[:])

            def bias_bcast(row_d, pool, psum_pool, tag):
                bv = pool.tile([PART, 2], F32, tag=f"biasv{tag}")
                nc.sync.dma_start(out=bv[:], in_=row_d[:, :].rearrange("c p -> p c"))
                bb = pool.tile([PART, HC], F32, tag=f"biasb{tag}")
                for c in range(2):
                    tp = psum_pool.tile([PART, PART], F32, tag=f"biastp{tag}{c}")
                    nc.tensor.transpose(out=tp[:], in_=bv[:, c:c + 1].to_broadcast([PART, PART]),
                                        identity=ident[:])
                    nc.vector.tensor_copy(out=bb[:, c * PART:(c + 1) * PART], in_=tp[:])
                return bb

            with tc.tile_pool(name="biasps", bufs=1, space="PSUM") as bps:
                b1b = bias_bcast(b1r_d, cpool, bps, "1")
                b2b = bias_bcast(b2r_d, cpool, bps, "2")

            # ---------- al pass L1: a_dst table for own dst nodes ----------
            with tc.tile_pool(name="alp", bufs=2) as ap_, \
                 tc.tile_pool(name="alw", bufs=1) as aw, \
                 tc.tile_pool(name="alps", bufs=2, space="PSUM") as aps:
                xT_sb = aw.tile([F_IN, NPC], F32)
                nc.scalar.dma_start(out=xT_sb[:], in_=xT_d[:, :])
                W1_sb = aw.tile([F_IN, 264], F32)
                nc.sync.dma_start(out=W1_sb[:], in_=Wcat1_d[:, :])
                alacc = aw.tile([PART, NB, 4], F32)
                for b in range(NB):
                    mb = min(PART, NPC - b * PART)
                    ps = aps.tile([PART, 8], F32, tag="alps")
                    nc.tensor.matmul(ps[0:mb, :], lhsT=xT_sb[:, b * PART:b * PART + mb],
                                     rhs=W1_sb[:, 256:264], start=True, stop=True)
                    nc.scalar.activation(out=alacc[0:mb, b, :], in_=ps[0:mb, 4:8],
                                         func=ACTF.Copy)
                nbf = NPC // PART            # 48 full blocks
                rem = NPC - nbf * PART       # 106
                nc.sync.dma_start(
                    out=al1[0:nbf * PART, 0:4].rearrange("(k p) w -> p k w", p=PART),
                    in_=alacc[:, 0:nbf, :])
                nc.sync.dma_start(out=al1[nbf * PART:NPC, 0:4], in_=alacc[0:rem, nbf, :])

                # ---------- dense L1 (replicated over all N nodes) ----------
                for g0 in range(0, NGB, DCH):
                    gcnt = min(DCH, NGB - g0)
                    n0 = g0 * PART
                    n1 = min(n0 + gcnt * PART, N)
                    xc = ap_.tile([F_IN, DCH * PART], F32, tag="d1x")
                    nc.scalar.dma_start(out=xc[:, 0:n1 - n0], in_=xTf_d[:, n0:n1])
                    stg = ap_.tile([PART, DCH, TW], BF16, tag="d1stg")
                    for j in range(gcnt):
                        mb = min(PART, N - (g0 + j) * PART)
                        ps = aps.tile([PART, 264], F32, tag="d1ps")
                        nc.tensor.matmul(ps[0:mb, :], lhsT=xc[:, j * PART:j * PART + mb],
                                         rhs=W1_sb[:], start=True, stop=True)
                        if j % 2 == 0:
                            nc.scalar.activation(out=stg[0:mb, j, 0:HC], in_=ps[0:mb, 0:HC],
                                                 func=ACTF.Copy)
                        else:
                            nc.vector.tensor_copy(out=stg[0:mb, j, 0:HC], in_=ps[0:mb, 0:HC])
                        nc.vector.tensor_copy(out=stg[0:mb, j, 256:264].bitcast(F32),
                                              in_=ps[0:mb, 256:260])
                    fullk = (n1 - n0) // PART
                    if fullk:
                        nc.gpsimd.dma_start(
                            out=T1[n0:n0 + fullk * PART, 0:264].rearrange(
                                "(k p) w -> p k w", p=PART),
                            in_=stg[:, 0:fullk, 0:264])
                    if (n1 - n0) % PART:
                        nc.gpsimd.dma_start(out=T1[n0 + fullk * PART:n1, 0:264],
                                            in_=stg[0:(n1 - n0) % PART, fullk, 0:264])

            # ---------- edge phase (shared for both layers) ----------
            def edge_phase(Tbl, altab, bbias, outT, relu, agout):
                with tc.tile_pool(name="eidx", bufs=2) as ip, \
                     tc.tile_pool(name="eg", bufs=2) as gp, \
                     tc.tile_pool(name="ew", bufs=2) as wp2, \
                     tc.tile_pool(name="eps", bufs=2, space="PSUM") as ep, \
                     tc.tile_pool(name="etps", bufs=2, space="PSUM") as tps:
                    for b in range(NB):
                        mb = min(PART, NPC - b * PART)
                        ia = ip.tile([PART, IW], I16, tag="ia")
                        nc.sync.dma_start(out=ia[:], in_=iall_d[b, :, :])
                        dl = ip.tile([PART, KT], F32, tag="dl")
                        nc.vector.tensor_copy(out=dl[:], in_=ia[:, 2 * KT * 8:])

                        # dma_gather is limited to 1024 indices per
                        # instruction -> chunk by 8 128-row groups.
                        def gather_chunks(gtile, src, i0, ktot, elem, koff=0):
                            for c0 in range(0, ktot, 8):
                                cnt = min(8, ktot - c0)
                                nc.gpsimd.dma_gather(
                                    out_ap=gtile[:, koff + c0:koff + c0 + cnt, :],
                                    in_ap=src,
                                    idxs_ap=ia[:, i0 + c0 * 8:i0 + (c0 + cnt) * 8],
                                    num_idxs=cnt * PART, num_idxs_reg=cnt * PART,
                                    elem_size=elem)

                        gall = gp.tile([PART, KT, TW], BF16, tag="gall")
                        gather_chunks(gall, Tbl[:, :], 0, KLO, TW)
                        gather_chunks(gall, Tbl[SPLIT:N, :], KLO * 8, KHI, TW, koff=KLO)
                        ga = gp.tile([PART, KT, ALW], F32, tag="ga")
                        gather_chunks(ga, altab[:, :], KT * 8, KT, ALW)

                        if EDGE_OPS == 1:
                            tok = wp2.tile([PART, 1], F32, tag="tok")
                            nc.vector.tensor_reduce(out=tok[:], in_=ga[:, 0, 0:4],
                                                    axis=mybir.AxisListType.X, op=ALU.add)
                            nc.sync.dma_start(out=outT[0:PART, b:b + 1], in_=tok[:])
                            continue

                        # per-edge logits z = a_src + a_dst; w = exp(lrelu(z))
                        Z = wp2.tile([PART, KT, 4], F32, tag="Z")
                        nc.vector.tensor_tensor(out=Z[:, :, :],
                                                in0=gall[:, :, 256:264].bitcast(F32),
                                                in1=ga[:, :, 0:4], op=ALU.add)
                        ZT = wp2.tile([PART, KT, 4], F32, tag="ZT")
                        nc.vector.tensor_scalar_mul(out=ZT[:], in0=Z[:], scalar1=NEG_SLOPE)
                        nc.vector.tensor_tensor(out=Z[:], in0=Z[:], in1=ZT[:], op=ALU.max)
                        EXb = wp2.tile([PART, KT, 4], BF16, tag="EXb")
                        nc.scalar.activation(out=EXb[:], in_=Z[:], func=ACTF.Exp)

                        # selector matrices: S01[p,k,d] = (dstloc[p,k]==d)
                        S01 = wp2.tile([PART, KT, PART], BF16, tag="S01")
                        nc.vector.tensor_tensor(
                            out=S01[:, :, :],
                            in0=dl[:, :].unsqueeze(-1).to_broadcast([PART, KT, PART]),
                            in1=iota_f[:].unsqueeze(1).to_broadcast([PART, KT, PART]),
                            op=ALU.is_equal)

                        # weighted messages [w*h | w]
                        Hp = wp2.tile([PART, KT, 260], BF16, tag="Hp")
                        nc.vector.tensor_tensor(
                            out=Hp[:, :, 0:HC].rearrange("p k (h c) -> p k h c", h=H),
                            in0=gall[:, :, 0:HC].rearrange("p k (h c) -> p k h c", h=H),
                            in1=EXb[:, :, :].unsqueeze(-1).to_broadcast([PART, KT, H, C0]),
                            op=ALU.mult)
                        nc.vector.tensor_copy(out=Hp[:, :, HC:HC + 4], in_=EXb[:])

                        # segment sum via selector matmuls
                        acc = ep.tile([PART, 260], F32, tag="acc")
                        for e in range(KT):
                            nc.tensor.matmul(acc[:], lhsT=S01[:, e, :], rhs=Hp[:, e, :],
                                             start=(e == 0), stop=(e == KT - 1))

                        dn = wp2.tile([PART, 4], F32, tag="dn")
                        nc.vector.tensor_scalar_add(out=dn[:], in0=acc[:, HC:HC + 4],
                                                    scalar1=1e-16)
                        rec = wp2.tile([PART, 4], F32, tag="rec")
                        nc.vector.reciprocal(out=rec[:], in_=dn[:])
                        ob = wp2.tile([PART, HC], F32, tag="ob")
                        nc.vector.tensor_tensor(
                            out=ob[:].rearrange("p (h c) -> p h c", h=H),
                            in0=acc[:, 0:HC].rearrange("p (h c) -> p h c", h=H),
                            in1=rec[:].unsqueeze(-1).to_broadcast([PART, H, C0]),
                            op=ALU.mult)
                        nc.vector.tensor_tensor(out=ob[:], in0=ob[:], in1=bbias[:], op=ALU.add)
                        if relu:
                            nc.vector.tensor_scalar_max(out=ob[:], in0=ob[:], scalar1=0.0)
                        tsb = wp2.tile([PART, 2, PART], F32, tag="tsb")
                        if agout is not None:
                            tsbb = wp2.tile([PART, 2, PART], BF16, tag="tsbb")
                        for c in range(2):
                            tp = tps.tile([PART, PART], F32, tag="ttp")
                            nc.tensor.transpose(out=tp[:], in_=ob[:, c * PART:(c + 1) * PART],
                                                identity=ident[:])
                            if c == 0:
                                nc.scalar.activation(out=tsb[:, c, :], in_=tp[:], func=ACTF.Copy)
                            else:
                                nc.vector.tensor_copy(out=tsb[:, c, :], in_=tp[:])
                            if agout is not None:
                                if c == 0:
                                    nc.vector.tensor_copy(out=tsbb[:, c, :], in_=tp[:])
                                else:
                                    nc.scalar.activation(out=tsbb[:, c, :], in_=tp[:],
                                                         func=ACTF.Copy)
                        nc.scalar.dma_start(
                            out=outT[:, b * PART:b * PART + mb].rearrange(
                                "(c p) m -> p c m", p=PART),
                            in_=tsb[:, :, 0:mb])
                        if agout is not None:
                            nc.scalar.dma_start(
                                out=agout[:, b * PART:b * PART + mb].rearrange(
                                    "(c p) m -> p c m", p=PART),
                                in_=tsbb[:, :, 0:mb])

            if PHASES < 2:
                raise _PhaseStopE
            edge_phase(T1, al1, b1b, g1T, relu=False, agout=agin)

            # ---------- BN stats (own slice) -> AllReduce; then AllGather ----
            if PHASES < 3:
                raise _PhaseStopE
            with tc.tile_pool(name="bnw", bufs=1) as bw, \
                 tc.tile_pool(name="gw", bufs=1) as gw:
                g1sb = [gw.tile([PART, NPC], F32, tag=f"g1sb{kt}", name=f"g1sb{kt}")
                        for kt in range(2)]
                stats = bw.tile([PART, 4], F32)
                with tc.tile_pool(name="st", bufs=1) as sp:
                    for ct in range(2):
                        nc.scalar.dma_start(out=g1sb[ct][:],
                                            in_=g1T[ct * PART:(ct + 1) * PART, :])
                        nc.vector.tensor_reduce(out=stats[:, ct:ct + 1], in_=g1sb[ct][:],
                                                axis=mybir.AxisListType.X, op=ALU.add)
                        sq = sp.tile([PART, NPC], F32, tag="sq")
                        nc.scalar.activation(out=sq[:], in_=g1sb[ct][:], func=ACTF.Square)
                        nc.vector.tensor_reduce(out=stats[:, 2 + ct:3 + ct], in_=sq[:],
                                                axis=mybir.AxisListType.X, op=ALU.add)
                    nc.sync.dma_start(out=ar_in[:, :], in_=stats[:])

                if not SKIP_COLL:
                    nc.gpsimd.collective_compute(
                        "AllReduce", ALU.add, replica_groups=rgroups,
                        ins=[ar_in[:, :]], outs=[ar_out[:, :]])
                    nc.gpsimd.collective_compute(
                        "AllGather", ALU.bypass, replica_groups=rgroups,
                        ins=[agin[:, :]], outs=[Gt_d[:, :]])

                ar_sb = bw.tile([PART, 4], F32)
                nc.sync.dma_start(out=ar_sb[:], in_=ar_out[:, :])
                mean = bw.tile([PART, 2], F32)
                nc.vector.tensor_scalar_mul(out=mean[:], in0=ar_sb[:, 0:2], scalar1=1.0 / N)
                msq = bw.tile([PART, 2], F32)
                nc.vector.tensor_scalar_mul(out=msq[:], in0=ar_sb[:, 2:4], scalar1=1.0 / N)
                var = bw.tile([PART, 2], F32)
                nc.vector.tensor_tensor(out=var[:], in0=mean[:], in1=mean[:], op=ALU.mult)
                nc.vector.tensor_tensor(out=var[:], in0=msq[:], in1=var[:], op=ALU.subtract)
                nc.vector.tensor_scalar_add(out=var[:], in0=var[:], scalar1=BN_EPS)
                sd = bw.tile([PART, 2], F32)
                nc.scalar.activation(out=sd[:], in_=var[:], func=ACTF.Sqrt)
                rinv = bw.tile([PART, 2], F32)
                nc.vector.reciprocal(out=rinv[:], in_=sd[:])
                gc = bw.tile([PART, 2], F32)
                nc.sync.dma_start(out=gc[:], in_=gcol_d[:, :])
                bc = bw.tile([PART, 2], F32)
                nc.sync.dma_start(out=bc[:], in_=bcol_d[:, :])
                scale_c = bw.tile([PART, 2], F32)
                nc.vector.tensor_tensor(out=scale_c[:], in0=gc[:], in1=rinv[:], op=ALU.mult)
                shift_c = bw.tile([PART, 2], F32)
                nc.vector.tensor_tensor(out=shift_c[:], in0=mean[:], in1=scale_c[:], op=ALU.mult)
                nc.vector.tensor_tensor(out=shift_c[:], in0=bc[:], in1=shift_c[:], op=ALU.subtract)

                if PHASES < 4:
                    raise _PhaseStopE

                # ---------- overlap region: own-node work during AllGather ----
                with tc.tile_pool(name="pl", bufs=1) as pl:
                    mk = pl.tile([PART, NPC], F32, tag="mk")
                    nc.sync.dma_start(out=mk[:], in_=mask_d[0:1, :].to_broadcast([PART, NPC]))
                    cv = pl.tile([PART, NPC], F32, tag="cv")
                    nc.sync.dma_start(out=cv[:], in_=cinv_d[0:1, :].to_broadcast([PART, NPC]))

                    with tc.tile_pool(name="ow", bufs=1) as ow, \
                         tc.tile_pool(name="owp", bufs=2, space="PSUM") as owp, \
                         tc.tile_pool(name="owb", bufs=1) as owb:
                        B2h = [ow.tile([PART, 8], F32, tag=f"b2h{kt}", name=f"b2h{kt}")
                               for kt in range(2)]
                        for kt in range(2):
                            nc.sync.dma_start(out=B2h[kt][:],
                                              in_=B2f_d[kt * PART:(kt + 1) * PART, :])
                        x1c = [ow.tile([PART, NPC], F32, tag=f"x1c{kt}", name=f"x1c{kt}")
                               for kt in range(2)]
                        for kt in range(2):
                            nc.scalar.activation(out=x1c[kt][:], in_=g1sb[kt][:],
                                                 func=ACTF.Relu,
                                                 bias=shift_c[:, kt:kt + 1],
                                                 scale=scale_c[:, kt:kt + 1])
                        alacc2 = ow.tile([PART, NB, 4], F32)
                        for b in range(NB):
                            mb = min(PART, NPC - b * PART)
                            ps = owp.tile([PART, 8], F32, tag="al2ps")
                            for kt in range(2):
                                nc.tensor.matmul(
                                    ps[0:mb, :],
                                    lhsT=x1c[kt][:, b * PART:b * PART + mb],
                                    rhs=B2h[kt][:], start=(kt == 0), stop=(kt == 1))
                            nc.scalar.activation(out=alacc2[0:mb, b, :], in_=ps[0:mb, 4:8],
                                                 func=ACTF.Copy)
                        nbf = NPC // PART
                        rem = NPC - nbf * PART
                        nc.sync.dma_start(
                            out=al2[0:nbf * PART, 0:4].rearrange("(k p) w -> p k w", p=PART),
                            in_=alacc2[:, 0:nbf, :])
                        nc.sync.dma_start(out=al2[nbf * PART:NPC, 0:4],
                                          in_=alacc2[0:rem, nbf, :])

                        # pooling of x1 channels (ct = 0, 1) from x1c
                        if PHASES >= 6:
                            for ct in range(2):
                                scr = g1sb[ct]
                                nc.vector.tensor_tensor_scan(
                                    out=scr[:], data0=mk[:], data1=x1c[ct][:],
                                    initial=0.0, op0=ALU.mult, op1=ALU.max)
                                nc.sync.dma_start(out=omax_d[ct * PART:(ct + 1) * PART, :],
                                                  in_=scr[:])
                                nc.vector.tensor_tensor_scan(
                                    out=scr[:], data0=mk[:], data1=x1c[ct][:],
                                    initial=0.0, op0=ALU.mult, op1=ALU.add)
                                nc.vector.tensor_tensor(out=scr[:], in0=scr[:], in1=cv[:],
                                                        op=ALU.mult)
                                nc.sync.dma_start(out=omean_d[ct * PART:(ct + 1) * PART, :],
                                                  in_=scr[:])

                    # ---------- dense L2 (replicated, from gathered table) ----
                    with tc.tile_pool(name="d2", bufs=2) as dp2, \
                         tc.tile_pool(name="d2w", bufs=1) as wp3, \
                         tc.tile_pool(name="d2ps", bufs=2, space="PSUM") as pp2:
                        W2_sb = [wp3.tile([PART, 264], BF16, tag=f"w2_{kt}", name=f"w2_{kt}")
                                 for kt in range(2)]
                        for kt in range(2):
                            nc.sync.dma_start(out=W2_sb[kt][:],
                                              in_=W2cat_d[kt * PART:(kt + 1) * PART, :])
                        for g0 in range(0, NGB, DCH):
                            gcnt = min(DCH, NGB - g0)
                            n0 = g0 * PART
                            n1 = min(n0 + gcnt * PART, N)
                            # chunk [n0, n1) may straddle core boundaries in Gt
                            segs = []
                            r0, r1 = n0 // NPC, (n1 - 1) // NPC
                            for r in range(r0, r1 + 1):
                                lo = max(n0, r * NPC); hi = min(n1, (r + 1) * NPC)
                                segs.append((r, lo - r * NPC, hi - lo, lo - n0))
                            x1s_l = []
                            for kt in range(2):
                                xg = dp2.tile([PART, DCH * PART], BF16, tag="d2xg")
                                for (r, c0, ln, xo) in segs:
                                    nc.scalar.dma_start(
                                        out=xg[:, xo:xo + ln],
                                        in_=Gt_d[r * HC + kt * PART:r * HC + (kt + 1) * PART,
                                                 c0:c0 + ln])
                                x1s = dp2.tile([PART, DCH * PART], BF16, tag="d2x1s")
                                nc.scalar.activation(out=x1s[:, 0:n1 - n0], in_=xg[:, 0:n1 - n0],
                                                     func=ACTF.Relu,
                                                     bias=shift_c[:, kt:kt + 1],
                                                     scale=scale_c[:, kt:kt + 1])
                                x1s_l.append(x1s)
                            stg = dp2.tile([PART, DCH, TW], BF16, tag="d2stg")
                            for j in range(gcnt):
                                mb = min(PART, N - (g0 + j) * PART)
                                ps = pp2.tile([PART, 264], F32, tag="d2ps")
                                for kt in range(2):
                                    nc.tensor.matmul(
                                        ps[0:mb, :],
                                        lhsT=x1s_l[kt][:, j * PART:j * PART + mb],
                                        rhs=W2_sb[kt][:], start=(kt == 0), stop=(kt == 1))
                                if j % 2 == 0:
                                    nc.scalar.activation(out=stg[0:mb, j, 0:HC],
                                                         in_=ps[0:mb, 0:HC], func=ACTF.Copy)
                                else:
                                    nc.vector.tensor_copy(out=stg[0:mb, j, 0:HC],
                                                          in_=ps[0:mb, 0:HC])
                                nc.vector.tensor_copy(out=stg[0:mb, j, 256:264].bitcast(F32),
                                                      in_=ps[0:mb, 256:260])
                            fullk = (n1 - n0) // PART
                            if fullk:
                                nc.gpsimd.dma_start(
                                    out=T2[n0:n0 + fullk * PART, 0:264].rearrange(
                                        "(k p) w -> p k w", p=PART),
                                    in_=stg[:, 0:fullk, 0:264])
                            if (n1 - n0) % PART:
                                nc.gpsimd.dma_start(out=T2[n0 + fullk * PART:n1, 0:264],
                                                    in_=stg[0:(n1 - n0) % PART, fullk, 0:264])

                    if PHASES < 5:
                        raise _PhaseStopE
                    edge_phase(T2, al2, b2b, x2T, relu=True, agout=None)

                    # ---------- pooling of x2 channels (ct = 2, 3) ----------
                    if PHASES < 6:
                        raise _PhaseStopE
                    with tc.tile_pool(name="pl2", bufs=1) as pl2:
                        for ct in range(2, 4):
                            xt = pl2.tile([PART, NPC], F32, tag="xt")
                            nc.scalar.dma_start(out=xt[:],
                                                in_=x2T[(ct - 2) * PART:(ct - 1) * PART, :])
                            sm = pl2.tile([PART, NPC], F32, tag="sm")
                            nc.vector.tensor_tensor_scan(out=sm[:], data0=mk[:], data1=xt[:],
                                                         initial=0.0, op0=ALU.mult, op1=ALU.max)
                            nc.sync.dma_start(out=omax_d[ct * PART:(ct + 1) * PART, :], in_=sm[:])
                            ss = pl2.tile([PART, NPC], F32, tag="ss")
                            nc.vector.tensor_tensor_scan(out=ss[:], data0=mk[:], data1=xt[:],
                                                         initial=0.0, op0=ALU.mult, op1=ALU.add)
                            nc.vector.tensor_tensor(out=ss[:], in0=ss[:], in1=cv[:], op=ALU.mult)
                            nc.sync.dma_start(out=omean_d[ct * PART:(ct + 1) * PART, :], in_=ss[:])

      except _PhaseStopE:
        pass

    nc.compile()
    return nc


# --------------------------------------------------------------------------
# host-side combine
# --------------------------------------------------------------------------

def postprocess(results, meta):
    lastcol = meta["lastcol"]
    mean = np.zeros((G, 2 * HC), np.float32)
    mx = np.zeros((G, 2 * HC), np.float32)
    for r in range(NCORES):
        om = results[r]["out_mean"]   # [512, NPC]
        ox = results[r]["out_max"]
        for g_, col in lastcol[r].items():
            mean[g_] += om[:, col]
            mx[g_] = np.maximum(mx[g_], ox[:, col])
    # empty graphs stay 0 (matches reference semantics)
    return np.concatenate([mean, mx], axis=1).astype(np.float32)


_CACHE = {}


def kernel(**inputs):
    in_maps, meta = preprocess(**inputs)
    key = (meta["NB"], meta["KLO"], meta["KHI"])
    if key not in _CACHE:
        _CACHE[key] = build_program(meta)
    nc = _CACHE[key]
    res = bass_utils.run_bass_kernel_spmd(nc, in_maps, core_ids=list(range(NCORES)))
    return postprocess(res.results, meta)

